# revision 8
# baseline (speedup 1.0000x reference)
"""Expert-parallel MoE kernel for Trainium2 (8 NeuronCores).

Sharding: core e owns expert e. The host computes the top-2 routing (in
float64) only to decide which token rows go to which core's shard; every
numerical value in the output is computed on device:
  - the gate (logits -> top-2 softmax weight for this core's expert) is
    recomputed on device from x and the replicated Wg/bg,
  - the expert MLP relu(x@W1+b1)@W2+b2 runs on device in float32r,
  - the per-token gate weight is applied on device.
The host gathers each expert's token rows (transposed, zero-padded to a
common capacity), runs the SPMD kernel, and scatter-adds the per-core
results into the full [T, D_OUT] output.

Device schedule: the hidden dimension is processed in NQ=4 resident-weight
passes (the W1/W2 quarter stays in SBUF across all token blocks), each pass
emitting a partial output; the host sums the partials. Within a pass,
token blocks of 1024 are processed with the stationary matmul operand
reused across two moving chunks to amortize the fp32r weight-load cost.
"""

import math
import os
import sys

import numpy as np

sys.path.insert(0, "/opt/trn_rl_repo")

P = 128
E = 8
DIN = 1024
DH = 4096
DO = 1024
KC = DIN // P   # 8  k-chunks of x / W1 contraction
HC = DH // P    # 32 h-chunks of W2 contraction
NQ = 4          # dh quarters (weight-resident passes)
HCQ = HC // NQ  # 8 h-chunks per quarter
NCORES = 8
TBMAX = 1024    # max tokens per block
NSBMAX = TBMAX // P
BIG = 1.0e30

_compiled = {}
LAST_DISPATCH_S = None


def _build(blocks, reps):
    import concourse.mybir as mybir
    import concourse.tile as tile
    from concourse import bacc

    F32 = mybir.dt.float32
    F32R = mybir.dt.float32r
    X = mybir.AxisListType.X

    cap = sum(blocks)
    S = cap // P

    nc = bacc.Bacc("TRN2", target_bir_lowering=False, debug=False,
                   num_devices=NCORES)

    xT = nc.dram_tensor("xT", [P, KC, cap], F32R, kind="ExternalInput").ap()
    W1m = nc.dram_tensor("W1m", [P, KC, DH], F32R, kind="ExternalInput").ap()
    W2m = nc.dram_tensor("W2m", [P, HC, DO], F32R, kind="ExternalInput").ap()
    Wgm = nc.dram_tensor("Wgm", [P, KC, E], F32R, kind="ExternalInput").ap()
    b1c = nc.dram_tensor("b1c", [P, HC], F32, kind="ExternalInput").ap()
    b2r = nc.dram_tensor("b2r", [P, DO], F32, kind="ExternalInput").ap()
    bgr = nc.dram_tensor("bgr", [P, E], F32, kind="ExternalInput").ap()
    sel4 = nc.dram_tensor("sel4", [P, NSBMAX, E], F32, kind="ExternalInput").ap()
    out = nc.dram_tensor("out", [NQ, S, P, DO], F32, kind="ExternalOutput").ap()

    with tile.TileContext(nc) as tc:
        with tc.tile_pool(name="const", bufs=1) as cpool, \
             tc.tile_pool(name="xtp", bufs=1) as xtp, \
             tc.tile_pool(name="w1p", bufs=2) as w1p, \
             tc.tile_pool(name="w2p", bufs=1) as w2p, \
             tc.tile_pool(name="htp", bufs=1) as htp, \
             tc.tile_pool(name="obp", bufs=4) as obp, \
             tc.tile_pool(name="gate", bufs=2) as gpool, \
             tc.tile_pool(name="ps", bufs=4, space="PSUM") as ps, \
             tc.tile_pool(name="psg", bufs=1, space="PSUM") as psg:

            wg_sb = cpool.tile([P, KC, E], F32R)
            nc.sync.dma_start(wg_sb[:], Wgm[:])
            bg_sb = cpool.tile([P, E], F32)
            nc.sync.dma_start(bg_sb[:], bgr[:])
            b1_sb = cpool.tile([P, HC], F32)
            nc.sync.dma_start(b1_sb[:], b1c[:])
            b2_sb = cpool.tile([P, DO], F32)
            nc.sync.dma_start(b2_sb[:], b2r[:])
            sel_sb = cpool.tile([P, NSBMAX, E], F32)
            nc.sync.dma_start(sel_sb[:], sel4[:])
            wcol_all = cpool.tile([P, S, 1], F32)

            def gate_block(xt, tb, s0):
                """This core's per-token gate weight for one block."""
                nsb = tb // P
                lgb = gpool.tile([P, NSBMAX, E], F32, tag="lgb",
                                 name="lgb")[:, :nsb]
                for s in range(nsb):
                    gps = psg.tile([P, E], F32, tag="gps", name="gps")
                    for kc in range(KC):
                        nc.tensor.matmul(
                            gps[:], xt[:, kc, s * P:(s + 1) * P],
                            wg_sb[:, kc, :],
                            start=(kc == 0), stop=(kc == KC - 1))
                    nc.vector.tensor_tensor(
                        lgb[:, s, :], gps[:], bg_sb[:], mybir.AluOpType.add)

                gw = gpool.tile([P, NSBMAX, 28], F32, tag="gw", name="gw")
                _c = [0]

                def g(w):
                    c = _c[0]
                    _c[0] += w
                    return gw[:, :nsb, c:c + w]

                m1 = g(1)
                nc.vector.reduce_max(m1[:], lgb[:], axis=X)
                eq = g(E)
                nc.vector.tensor_tensor(eq[:], lgb[:],
                                        m1.to_broadcast([P, nsb, E]),
                                        mybir.AluOpType.is_ge)
                cnt = g(1)
                nc.vector.reduce_sum(cnt[:], eq[:], axis=X)
                tmp = g(E)
                nc.vector.tensor_scalar_mul(tmp[:], eq[:], BIG)
                nc.vector.tensor_sub(tmp[:], lgb[:], tmp[:])
                m2 = g(1)
                nc.vector.reduce_max(m2[:], tmp[:], axis=X)
                msk = g(1)
                nc.vector.tensor_scalar(msk[:], cnt[:], 2.0, None,
                                        mybir.AluOpType.is_ge)
                dd = g(1)
                nc.vector.tensor_sub(dd[:], m1[:], m2[:])
                nc.vector.tensor_tensor(dd[:], dd[:], msk[:],
                                        mybir.AluOpType.mult)
                nc.vector.tensor_add(m2[:], m2[:], dd[:])
                lsel = g(1)
                wst = gpool.tile([P, NSBMAX, E], F32, tag="wst",
                                 name="wst")[:, :nsb]
                nc.vector.tensor_tensor(wst[:], lgb[:], sel_sb[:, :nsb],
                                        mybir.AluOpType.mult)
                nc.vector.reduce_sum(lsel[:], wst[:], axis=X)
                d2 = g(1)
                nc.vector.tensor_sub(d2[:], m2[:], m1[:])
                e2 = g(1)
                nc.scalar.activation(e2[:], d2[:],
                                     mybir.ActivationFunctionType.Exp)
                den = g(1)
                nc.vector.tensor_scalar_add(den[:], e2[:], 1.0)
                rec = g(1)
                nc.vector.reciprocal(rec[:], den[:])
                dsel = g(1)
                nc.vector.tensor_sub(dsel[:], lsel[:], m1[:])
                wex = g(1)
                nc.scalar.activation(wex[:], dsel[:],
                                     mybir.ActivationFunctionType.Exp)
                nc.vector.tensor_tensor(wcol_all[:, s0:s0 + nsb], wex[:],
                                        rec[:], mybir.AluOpType.mult)

            def body(_iv=None):
                for q in range(NQ):
                    w1q = w1p.tile([P, KC, HCQ * P], F32R, tag="w1q",
                                   name="w1q")
                    nc.sync.dma_start(
                        w1q[:], W1m[:, :, q * HCQ * P:(q + 1) * HCQ * P])
                    w2q = w2p.tile([P, HCQ, DO], F32R, tag="w2q", name="w2q")
                    nc.sync.dma_start(
                        w2q[:], W2m[:, q * HCQ:(q + 1) * HCQ, :])

                    s0 = 0
                    for tb in blocks:
                        nsb = tb // P
                        t0 = s0 * P
                        ntch = (tb + 511) // 512

                        xt = xtp.tile([P, KC, TBMAX], F32R, tag="xt",
                                      name="xt")[:, :, :tb]
                        nc.sync.dma_start(xt[:], xT[:, :, t0:t0 + tb])

                        if q == 0:
                            gate_block(xt, tb, s0)

                        # layer 1 (quarter): hTq = relu(W1q.T @ x + b1q)
                        hTq = htp.tile([P, HCQ, TBMAX], F32R, tag="hTq",
                                       name="hTq")[:, :, :tb]
                        for hcl in range(HCQ):
                            pts = [
                                ps.tile([P, 512], F32, tag="mm", name="mm")
                                for _ in range(ntch)
                            ]
                            for kc in range(KC):
                                for tch in range(ntch):
                                    w = min(512, tb - tch * 512)
                                    nc.tensor.matmul(
                                        pts[tch][:, :w],
                                        w1q[:, kc, hcl * P:(hcl + 1) * P],
                                        xt[:, kc, tch * 512:tch * 512 + w],
                                        start=(kc == 0), stop=(kc == KC - 1))
                            for tch in range(ntch):
                                w = min(512, tb - tch * 512)
                                nc.scalar.activation(
                                    hTq[:, hcl, tch * 512:tch * 512 + w],
                                    pts[tch][:, :w],
                                    mybir.ActivationFunctionType.Relu,
                                    bias=b1_sb[:, q * HCQ + hcl:
                                               q * HCQ + hcl + 1],
                                    scale=1.0)

                        # layer 2 (quarter partial): out_q = hTq.T @ W2q
                        for s in range(nsb):
                            p2 = [
                                ps.tile([P, 512], F32, tag="mm", name="mm")
                                for _ in range(2)
                            ]
                            for hcl in range(HCQ):
                                for dt in range(2):
                                    nc.tensor.matmul(
                                        p2[dt][:],
                                        hTq[:, hcl, s * P:(s + 1) * P],
                                        w2q[:, hcl, dt * 512:(dt + 1) * 512],
                                        start=(hcl == 0),
                                        stop=(hcl == HCQ - 1))
                            ob = obp.tile([P, DO], F32, tag="ob", name="ob")
                            for dt in range(2):
                                if q == 0:
                                    nc.vector.tensor_add(
                                        ob[:, dt * 512:(dt + 1) * 512],
                                        p2[dt][:],
                                        b2_sb[:, dt * 512:(dt + 1) * 512])
                                else:
                                    nc.vector.tensor_scalar_mul(
                                        ob[:, dt * 512:(dt + 1) * 512],
                                        p2[dt][:],
                                        wcol_all[:, s0 + s, :])
                            if q == 0:
                                nc.vector.tensor_scalar_mul(
                                    ob[:], ob[:], wcol_all[:, s0 + s, :])
                            nc.sync.dma_start(out[q, s0 + s], ob[:])
                        s0 += nsb

            if reps > 1:
                with tc.For_i(0, reps, 1) as _i:
                    body(_i)
            else:
                body()

    nc.compile()
    return nc


def _get_compiled(blocks, reps):
    key = (tuple(blocks), reps)
    if key not in _compiled:
        _compiled[key] = _build(blocks, reps)
    return _compiled[key]


def kernel(x, Wg, bg, W1, b1, W2, b2):
    import time as _time

    from concourse.bass_utils import run_bass_kernel_spmd

    x = np.ascontiguousarray(np.asarray(x, dtype=np.float32))
    Wg = np.ascontiguousarray(np.asarray(Wg, dtype=np.float32))
    bg = np.ascontiguousarray(np.asarray(bg, dtype=np.float32))
    W1 = np.ascontiguousarray(np.asarray(W1, dtype=np.float32))
    b1 = np.ascontiguousarray(np.asarray(b1, dtype=np.float32))
    W2 = np.ascontiguousarray(np.asarray(W2, dtype=np.float32))
    b2 = np.ascontiguousarray(np.asarray(b2, dtype=np.float32))

    T = x.shape[0]

    # Host-side routing (float64) decides the shards only.
    logits = x.astype(np.float64) @ Wg.astype(np.float64) + bg.astype(np.float64)
    top2 = np.argpartition(logits, -2, axis=1)[:, -2:]
    sel_mask = np.zeros((T, E), dtype=bool)
    sel_mask[np.arange(T)[:, None], top2] = True

    idx_e = [np.nonzero(sel_mask[:, e])[0] for e in range(E)]
    counts = [len(i) for i in idx_e]
    cap = max(P, int(math.ceil(max(counts) / P)) * P)
    nfull, rem = divmod(cap, TBMAX)
    blocks = [TBMAX] * nfull + ([rem] if rem else [])

    reps = int(os.environ.get("MOE_REPS", "1"))
    nc = _get_compiled(blocks, reps)

    Wgm = Wg.reshape(KC, P, E).transpose(1, 0, 2).copy()
    bgr = np.tile(bg, (P, 1))

    in_maps = []
    for e in range(E):
        n = counts[e]
        xe = np.zeros((cap, DIN), dtype=np.float32)
        xe[:n] = x[idx_e[e]]
        sel = np.zeros(E, dtype=np.float32)
        sel[e] = 1.0
        in_maps.append({
            "xT": np.ascontiguousarray(
                xe.T.reshape(KC, P, cap).transpose(1, 0, 2)),
            "W1m": np.ascontiguousarray(
                W1[e].reshape(KC, P, DH).transpose(1, 0, 2)),
            "W2m": np.ascontiguousarray(
                W2[e].reshape(HC, P, DO).transpose(1, 0, 2)),
            "Wgm": Wgm,
            "b1c": np.ascontiguousarray(b1[e].reshape(HC, P).T),
            "b2r": np.tile(b2[e], (P, 1)),
            "bgr": bgr,
            "sel4": np.tile(sel, (P, NSBMAX, 1)),
        })

    _t0 = _time.time()
    res = run_bass_kernel_spmd(nc, in_maps, list(range(NCORES)))
    global LAST_DISPATCH_S
    LAST_DISPATCH_S = _time.time() - _t0

    outf = np.zeros((T, DO), dtype=np.float64)
    for e in range(E):
        oe = res.results[e]["out"].astype(np.float64)      # [NQ, S, P, DO]
        oe = oe.sum(axis=0).reshape(cap, DO)
        outf[idx_e[e]] += oe[:counts[e]]
    return outf.astype(np.float32)
